# revision 32
# baseline (speedup 1.0000x reference)
"""Trainium2 Bass kernel for nn_Block_50706383897045 (dense transformer block).

Strategy: data-parallel over batch - B=8 equals n_cores=8, one batch element
per core, no collectives. Per core the full block (LN -> QKV -> causal
attention -> out-proj -> residual -> LN -> MLP(gelu) -> residual) runs on a
[T=1024, C=768] slice.

v2 design notes:
- Host prep: x pre-transposed to feature-major (no on-chip transposes);
  ln_w/ln_b folded into w_qkv/w_c1 (both LNs share params); k-bias dropped
  (softmax shift invariance); v-bias folded into b_out (attn rows sum to 1);
  weights cast to bf16 (halves DMA + SBUF).
- LN: stats via ones-matmuls on PE, rstd via ACT Sqrt + DVE approx
  reciprocal, mean/rstd broadcast via K=1 PE matmuls in f32r (bf16 rows
  here would scale h per-token by ~0.4% and get exp-amplified to ~30%
  attention-weight error - keep them f32r), normalize = 2 DVE ops.
- Attention: h/k/q/scores in f32r (precision: score errors multiply
  through exp); causal trim (partial diagonal tiles use reduced moving
  width); the two heads of a feature tile issue score matmuls adjacently
  -> PE row-group concurrency (K=64, tile_position from base partitions);
  one exp per kt covers both heads ([128,2,CH] PSUM, adjacent banks);
  exp -> bf16; mask = one [128,128] DVE multiply per diagonal tile;
  denominator from a ones-column in V; reciprocal via DVE approx op
  (must read from SBUF, not PSUM); broadcast via gpsimd; attn@v bf16.
  KQ(f+1) software-pipelined under attention(f).
- PSUM evacuations fused with bias+residual via scalar_tensor_tensor.
- v1 1070785ns (harness) / 634us (local) -> v2 365620ns local.
"""
import sys

sys.path.insert(0, "/opt/trn_rl_repo")

import numpy as np
import ml_dtypes

import concourse.bass as bass
import concourse.bacc as bacc
import concourse.mybir as mybir
import concourse.tile as tile
from concourse import bass_utils

AF = mybir.ActivationFunctionType
ALU = mybir.AluOpType
f32 = mybir.dt.float32
f32r = mybir.dt.float32r
bf16 = mybir.dt.bfloat16
f8 = mybir.dt.float8e4
DR = mybir.MatmulPerfMode.DoubleRow

# PE throughput is 1 moving-row/cycle regardless of dtype; fp8 DoubleRow's
# win is K=256 per pass (half the row-passes), so only PURE fp8 (no hi/lo
# residual) is faster than bf16. Error budget allows it on part of c2 only:
# the first C2_U8 k-pairs (of 12) of c2 run fp8-DR (g + w_c2 quantized to
# e4m3, ~2% rms each); the rest stays bf16. All c2 weights are scaled x32
# (fp8 subnormal avoidance), descaled in the evacuation.
C1DR = False  # hi/lo c1 measured: same speed as bf16, extra noise. Keep off.
C2_U8 = 12    # k-pairs of c2 in fp8-DR (0..12); 12 = full fp8 c2

B, T, C, H, D = 8, 1024, 768, 12, 64
F = C // 128      # 6 feature tiles of the residual stream
NT = T // 128     # 8 token tiles
CH = 512          # token chunk (fp32 moving-operand max)
NCH = T // CH     # 2
M3 = 4 * C        # 3072 MLP hidden
EPS = 1e-5

_NC_CACHE = None


def _chunk(c):
    return slice(c * CH, (c + 1) * CH)


def _ln(nc, tc, pools, src, dst, ones_col, ones_row, eps_t, sq_on_act=True,
        chunks=None):
    """LayerNorm (params pre-folded into weights): dst = (src-mu)*rstd.
    src f32r [128, F, T], dst bf16 [128, F, T]. Stats via ones-matmuls,
    rstd via ACT Rsqrt, broadcast via K=1 PE matmuls."""
    ln_ps, sq_pool, row_pool, tmp_pool = pools
    for c in (range(NCH) if chunks is None else chunks):
        sl = _chunk(c)
        ps_sum = ln_ps.tile([1, CH], f32, tag="lnsum", name="ps_sum")
        ps_sq = ln_ps.tile([1, CH], f32, tag="lnsq", name="ps_sq")
        hc = CH // 2
        for f in range(F):
            # square split ACT/DVE halves so sq_t is ready ~2x sooner
            sq_t = sq_pool.tile([128, CH], f32r, tag="ln_sq", name="sq_t")
            nc.scalar.activation(sq_t[:, 0:hc], src[:, f, sl][:, 0:hc],
                                 AF.Square)
            nc.vector.tensor_mul(sq_t[:, hc:CH], src[:, f, sl][:, hc:CH],
                                 src[:, f, sl][:, hc:CH])
            nc.tensor.matmul(ps_sum, ones_col, src[:, f, sl],
                             start=(f == 0), stop=(f == F - 1))
            nc.tensor.matmul(ps_sq, ones_col, sq_t,
                             start=(f == 0), stop=(f == F - 1))
        mean = row_pool.tile([1, CH], f32, tag="ln_ra", name="mean")
        nc.vector.tensor_scalar_mul(mean, ps_sum, 1.0 / C)
        musq = row_pool.tile([1, CH], f32, tag="ln_rb", name="musq")
        nc.vector.tensor_mul(musq, mean, mean)
        var = row_pool.tile([1, CH], f32, tag="ln_rc", name="var")
        nc.vector.scalar_tensor_tensor(
            var, ps_sq, 1.0 / C, musq, ALU.mult, ALU.subtract)
        std = row_pool.tile([1, CH], f32, tag="ln_rb", name="std")
        nc.scalar.activation(std, var, AF.Sqrt, bias=eps_t)
        rstd = row_pool.tile([1, CH], f32, tag="ln_rc", name="rstd")
        nc.vector.reciprocal_approx_fast(out=rstd, in_=std)
        nmrs = row_pool.tile([1, CH], f32r, tag="ln_nm", name="nmrs")
        nc.vector.scalar_tensor_tensor(
            nmrs, mean, -1.0, rstd, ALU.mult, ALU.mult)
        rstd_b = row_pool.tile([1, CH], f32r, tag="ln_rb", name="rstd_b")
        nc.vector.tensor_copy(rstd_b, rstd)
        ps_rs = ln_ps.tile([128, CH], f32, tag="lnbc_rs", name="ps_rs")
        nc.tensor.matmul(ps_rs, ones_row, rstd_b, start=True, stop=True)
        ps_nm = ln_ps.tile([128, CH], f32, tag="lnbc_nm", name="ps_nm")
        nc.tensor.matmul(ps_nm, ones_row, nmrs, start=True, stop=True)
        # evacuate broadcasts to SBUF (GpSimd can't read PSUM; DVE SBUF
        # reads are cheaper than PSUM anyway)
        rs_sb = row_pool.tile([128, CH], f32, tag="ln_rssb", name="rs_sb")
        nc.vector.tensor_copy(rs_sb, ps_rs)
        nm_sb = row_pool.tile([128, CH], f32, tag="ln_nmsb", name="nm_sb")
        nc.scalar.copy(nm_sb, ps_nm)
        for f in range(F):
            # normalize split across DVE and GpSimd (Pool is idle here)
            eng = nc.vector if f % 2 == 0 else nc.gpsimd
            tmp = tmp_pool.tile([128, CH], f32, tag="ln_tmp", name="tmp")
            eng.tensor_mul(tmp, src[:, f, sl].bitcast(f32), rs_sb)
            eng.tensor_add(dst[:, f, sl], tmp, nm_sb)


def _build(debug_stage=None):
    nc = bacc.Bacc("TRN2", target_bir_lowering=False, debug=False,
                   num_devices=8)

    xT_d = nc.dram_tensor("xT", [C, T], f32, kind="ExternalInput")
    wqkv_d = nc.dram_tensor("w_qkv", [C, 3 * C], f32, kind="ExternalInput")
    wout_d = nc.dram_tensor("w_out", [C, C], bf16, kind="ExternalInput")
    if C1DR:
        # k-pair layout: [3*128, 2*M3]; row j*128+p, col i*M3+m holds
        # w_c1[(2j+i)*128+p, m] * 16 (hi) / its fp8 residual (lo)
        wc1_d = nc.dram_tensor("w_c1h", [3 * 128, 2 * M3], f8,
                               kind="ExternalInput")
        wc1l_d = nc.dram_tensor("w_c1l", [3 * 128, 2 * M3], f8,
                                kind="ExternalInput")
    else:
        wc1_d = nc.dram_tensor("w_c1", [C, M3], bf16, kind="ExternalInput")
        wc1l_d = None
    # c2 weights: first C2_U8 k-pairs as fp8 pair-layout, rest bf16 rows;
    # both hold w_c2 * 32
    wc2_d = (nc.dram_tensor("w_c2f8", [C2_U8 * 128, 2 * C], f8,
                            kind="ExternalInput") if C2_U8 else None)
    wc2l_d = (nc.dram_tensor("w_c2bf", [(24 - 2 * C2_U8) * 128, C], bf16,
                             kind="ExternalInput") if C2_U8 < 12 else None)
    bq_d = nc.dram_tensor("bq_pc", [128, F], f32, kind="ExternalInput")
    bout_d = nc.dram_tensor("bout_pc", [128, F], f32, kind="ExternalInput")
    bc1_d = nc.dram_tensor("bc1_pc", [128, 24], f32, kind="ExternalInput")
    bc2_d = nc.dram_tensor("bc2_pc", [128, F], f32, kind="ExternalInput")
    yT_d = nc.dram_tensor("yT", [C, T], f32, kind="ExternalOutput")
    yD_d = (nc.dram_tensor("yD", [C, T], f32, kind="ExternalOutput")
            if debug_stage else None)

    with tile.TileContext(nc) as tc:
        _kernel_body(nc, tc, xT_d, wqkv_d, wout_d, wc1_d, wc1l_d,
                     wc2_d, wc2l_d, bq_d, bout_d, bc1_d, bc2_d, yT_d,
                     debug_stage, yD_d)
    nc.compile()
    return nc


def _debug_dump(nc, tc, src_t, yD_d, cast=True):
    """Copy a [128, F, T] tile to the yD debug output."""
    with tc.tile_pool(name="dbg", bufs=2) as dbg_pool:
        for ct in range(F):
            for c in range(NCH):
                sl = _chunk(c)
                t = dbg_pool.tile([128, CH], f32, tag="dbg", name="dbg")
                nc.vector.tensor_copy(t, src_t[:, ct, sl])
                nc.sync.dma_start(
                    yD_d.ap()[ct * 128:(ct + 1) * 128, sl], t)


def _kernel_body(nc, tc, xT_d, wqkv_d, wout_d, wc1_d, wc1l_d,
                 wc2_d, wc2l_d, bq_d, bout_d, bc1_d, bc2_d, yT_d,
                 debug_stage=None, yD_d=None):
    with tc.tile_pool(name="persist", bufs=1) as persist:
        ones_col = persist.tile([128, 1], f32r)
        nc.vector.memset(ones_col.bitcast(f32), 1.0)
        ones_row = persist.tile([1, 128], f32r)
        nc.vector.memset(ones_row.bitcast(f32), 1.0)
        eps_t = persist.tile([1, 1], f32)
        nc.vector.memset(eps_t, EPS)
        # lower-triangular keep mask (tri[p, q] = 1 iff q >= p)
        tri = persist.tile([128, 128], bf16)
        with tc.tile_pool(name="trif", bufs=1) as trif_pool:
            tri_f = trif_pool.tile([128, 128], f32)
            nc.vector.memset(tri_f, 1.0)
            nc.gpsimd.affine_select(
                out=tri_f, in_=tri_f, compare_op=ALU.is_ge, fill=0.0,
                base=0, pattern=[[1, 128]], channel_multiplier=-1)
            nc.vector.tensor_copy(tri, tri_f)
        bq_c = persist.tile([128, F], f32)
        bout_c = persist.tile([128, F], f32)
        bc1_c = persist.tile([128, 24], f32)
        bc2_c = persist.tile([128, F], f32)


        with (
            tc.tile_pool(name="resid", bufs=1) as resid_pool,
            tc.tile_pool(name="hpool", bufs=1) as h_pool,
            tc.tile_pool(name="aopool", bufs=1) as ao_pool,
            tc.tile_pool(name="woutp", bufs=1) as wout_pool,
            tc.tile_pool(name="wc1p", bufs=1) as wc1_pool,
        ):
            x_fm = resid_pool.tile([128, F, T], f32r, tag="x_slot",
                                   name="x_fm")
            h_fm = h_pool.tile([128, F, T], f32r, tag="h_slot", name="h_fm")
            attn_out = ao_pool.tile([128, F, T], bf16, tag="attn_out",
                                    name="attn_out")

            # ---- input + weight DMAs (issued up front, in need order) ----
            for c in range(NCH):
                for f in range(F):
                    nc.sync.dma_start(
                        x_fm[:, f, _chunk(c)],
                        xT_d.ap().bitcast(f32r)
                        [f * 128:(f + 1) * 128, _chunk(c)])
            nc.sync.dma_start(bq_c, bq_d.ap())
            nc.sync.dma_start(bout_c, bout_d.ap())
            nc.sync.dma_start(bc1_c, bc1_d.ap())
            nc.sync.dma_start(bc2_c, bc2_d.ap())

            with (
                tc.tile_pool(name="wkq", bufs=1) as wkq_pool,
                tc.tile_pool(name="v1pool", bufs=1) as v1_pool,
            ):
                wv_pool = tc.alloc_tile_pool(name="wv", bufs=1)
                wv_t, wkq_t, wout_t, wc1_t = [], [], [], []
                # DMA issue order = need order: wv (~45us), wkq (~75us),
                # wout (~215us), wc1 (~230us)
                for kt in range(F):
                    wt = wv_pool.tile([128, C], f32r, tag=f"wv{kt}",
                                      name=f"wv{kt}")
                    nc.sync.dma_start(
                        wt, wqkv_d.ap().bitcast(f32r)
                        [kt * 128:(kt + 1) * 128, 2 * C:3 * C])
                    wv_t.append(wt)
                for kt in range(F):
                    wt = wkq_pool.tile([128, 2 * C], f32r, tag=f"wkq{kt}",
                                       name=f"wkq{kt}")
                    nc.sync.dma_start(
                        wt, wqkv_d.ap().bitcast(f32r)
                        [kt * 128:(kt + 1) * 128, 0:2 * C])
                    wkq_t.append(wt)
                for kt in range(F):
                    wt = wout_pool.tile([128, C], bf16, tag=f"wout{kt}",
                                        name=f"wout{kt}")
                    nc.sync.dma_start(
                        wt, wout_d.ap()[kt * 128:(kt + 1) * 128, :])
                    wout_t.append(wt)
                if C1DR:
                    for j in range(3):
                        for lbl, dram in (("h", wc1_d), ("l", wc1l_d)):
                            wt = wc1_pool.tile(
                                [128, 2 * M3], f8, tag=f"wc1{lbl}{j}",
                                name=f"wc1{lbl}{j}")
                            nc.sync.dma_start(
                                wt, dram.ap()[j * 128:(j + 1) * 128, :])
                            wc1_t.append(wt)
                else:
                    for kt in range(F):
                        wt = wc1_pool.tile([128, M3], bf16, tag=f"wc1{kt}",
                                           name=f"wc1{kt}")
                        nc.sync.dma_start(
                            wt, wc1_d.ap()[kt * 128:(kt + 1) * 128, :])
                        wc1_t.append(wt)

                # V with appended ones column per head (softmax denominator)
                v1 = v1_pool.tile([128, NT, H * 65], bf16, tag="v1",
                                  name="v1")
                nc.vector.memset(
                    v1.rearrange("p t (h m) -> p t h m", m=65)
                    [:, :, :, 64:65], 1.0)

                # ---- LN1 then V ----
                with (
                    tc.tile_pool(name="lnps", bufs=1, space="PSUM") as ln_ps,
                    tc.tile_pool(name="ln_sq", bufs=1) as sq_pool,
                    tc.tile_pool(name="ln_rows", bufs=1) as row_pool,
                    tc.tile_pool(name="ln_tmp", bufs=2) as tmp_pool,
                    tc.tile_pool(name="vps", bufs=3, space="PSUM") as v_ps,
                ):
                    with nc.named_scope("ln1"):
                        _ln(nc, tc, (ln_ps, sq_pool, row_pool,
                             tmp_pool),
                            x_fm, h_fm, ones_col, ones_row, eps_t)
                    with nc.named_scope("qkv_v"):
                        for t in range(NT):  # noqa: E501
                            for half in range(2):
                                ps_v = v_ps.tile([128, 384], f32, tag="vps",
                                                 name="ps_v")
                                c0 = half * 384
                                for kt in range(F):
                                    nc.tensor.matmul(
                                        ps_v,
                                        h_fm[:, kt, t * 128:(t + 1) * 128],
                                        wv_t[kt][:, c0:c0 + 384],
                                        start=(kt == 0), stop=(kt == F - 1))
                                dst = (v1[:, t, :]
                                       .rearrange("p (h m) -> p h m", m=65)
                                       [:, half * 6:(half + 1) * 6, 0:64])
                                nc.scalar.copy(
                                    dst,
                                    ps_v.rearrange("p (h m) -> p h m",
                                                   m=64))
                wv_pool.release()

                # ---- per feature tile: K,Q then attention (both chunks)
                with (
                    tc.tile_pool(name="kqf", bufs=2) as kqf_pool,
                    tc.tile_pool(name="sps", bufs=3, space="PSUM") as s_ps,
                    tc.tile_pool(name="yps", bufs=1, space="PSUM") as y_ps,
                    tc.tile_pool(name="bcps", bufs=1, space="PSUM") as bc_ps,
                    tc.tile_pool(name="expp", bufs=4) as exp_pool,
                    tc.tile_pool(name="rrow", bufs=1) as rr_pool,
                    tc.tile_pool(name="bcsb", bufs=1) as bcsb_pool,
                ):
                    def kq_phase(f):
                        """Emit K,Q matmuls + evacs for feature tile f."""
                        kf = kqf_pool.tile([128, T], f32r, tag="kf",
                                           name=f"kf{f}")
                        qf = kqf_pool.tile([128, T], f32r, tag="qf",
                                           name=f"qf{f}")
                        with nc.named_scope(f"kq_{f}"):
                            for dst_t, col0, isq in (
                                    (kf, f * 128, False),
                                    (qf, C + f * 128, True)):
                                for c in range(NCH):
                                    sl = _chunk(c)
                                    ps = s_ps.tile([128, 2, CH], f32,
                                                   tag="s", name="kq_ps")
                                    ps = ps[:, 0, :]
                                    for kt in range(F):
                                        nc.tensor.matmul(
                                            ps,
                                            wkq_t[kt][:, col0:col0 + 128],
                                            h_fm[:, kt, sl],
                                            start=(kt == 0),
                                            stop=(kt == F - 1))
                                    if isq:
                                        nc.vector.tensor_scalar_add(
                                            dst_t[:, sl],
                                            ps, bq_c[:, f:f + 1])
                                    else:
                                        nc.scalar.copy(
                                            dst_t[:, sl], ps)
                        return kf, qf

                    # software pipeline: KQ(f+1) issues before attn(f) so
                    # its PE matmuls cover the kf/qf evacuation latency
                    kqf_t = kq_phase(0)
                    for f in range(F):
                        nxt = kq_phase(f + 1) if f + 1 < F else None
                        kf, qf = kqf_t
                        for c in range(NCH):
                            with nc.named_scope(f"attn_f{f}_c{c}"):
                                _attn_pair(nc, (s_ps, y_ps, bc_ps, exp_pool,
                                                rr_pool, bcsb_pool),
                                           kf, qf, v1, attn_out,
                                           tri, ones_row, f, c)
                        kqf_t = nxt

            if debug_stage == 'h':
                _debug_dump(nc, tc, h_fm, yD_d)
            if debug_stage == 'attn':
                _debug_dump(nc, tc, attn_out, yD_d)

            x2_pool = tc.alloc_tile_pool(name="x2p", bufs=1)
            x2_fm = x2_pool.tile([128, F, T], f32r, tag="x2",
                                 name="x2_fm")
            h2_fm = h_pool.tile([128, F, T], f8 if C1DR else bf16,
                                tag="h_slot", name="h2_fm")
            with (
                tc.tile_pool(name="ops2", bufs=3, space="PSUM") as o2_ps,
                tc.tile_pool(name="ln2ps", bufs=1, space="PSUM") as ln2_ps,
                tc.tile_pool(name="ln2_sq", bufs=1) as sq2_pool,
                tc.tile_pool(name="ln2_rows", bufs=1) as row2_pool,
                tc.tile_pool(name="ln2_tmp", bufs=2) as tmp2_pool,
            ):
                for c in range(NCH):
                    sl = _chunk(c)
                    with nc.named_scope(f"out_proj_c{c}"):
                        for ct in range(F):
                            ps = o2_ps.tile([128, CH], f32, tag="o",
                                            name="o_ps")
                            for kt in range(F):
                                nc.tensor.matmul(
                                    ps,
                                    wout_t[kt][:, ct * 128:(ct + 1) * 128],
                                    attn_out[:, kt, sl],
                                    start=(kt == 0), stop=(kt == F - 1))
                            nc.vector.scalar_tensor_tensor(
                                x2_fm[:, ct, sl], ps,
                                bout_c[:, ct:ct + 1],
                                x_fm[:, ct, sl].bitcast(f32),
                                ALU.add, ALU.add)
                    with nc.named_scope(f"ln2_c{c}"):
                        _ln(nc, tc, (ln2_ps, sq2_pool, row2_pool,
                             tmp2_pool),
                            x2_fm, h2_fm, ones_col, ones_row, eps_t,
                            sq_on_act=False, chunks=[c])

            if True:
                if debug_stage == 'x2':
                    _debug_dump(nc, tc, x2_fm, yD_d)

                if debug_stage == 'h2':
                    _debug_dump(nc, tc, h2_fm, yD_d)

                # ---- MLP ----
                out_fm = resid_pool.tile([128, F, T], f32, tag="x_slot",
                                         name="out_fm")
                _mlp(nc, tc, wc1_t, wc2_d, wc2l_d, h2_fm, x2_fm,
                     out_fm, bc1_c, bc2_c, yT_d)
            x2_pool.release()


def _c1_matmuls(nc, ps_g, wc1_t, h2_fm, mt, sl):
    """Emit the c1 accumulation group for output tile mt."""
    if C1DR:
        # wc1_t = [h0, l0, h1, l1, h2, l2]; ps = sum_j Whi_j.T2 @ h_j
        # + sum_j Wlo_j.T2 @ h_j  (DoubleRow, K=256 per matmul)
        n = 6
        i = 0
        for lbl in range(2):  # hi then lo
            for j in range(3):
                wt = wc1_t[2 * j + lbl]
                lhs = wt.rearrange("p (a m) -> p a m", a=2)[
                    :, :, mt * 128:(mt + 1) * 128]
                nc.tensor.matmul(
                    ps_g, lhs, h2_fm[:, 2 * j:2 * j + 2, sl],
                    start=(i == 0), stop=(i == n - 1), perf_mode=DR)
                i += 1
    else:
        for kt in range(F):
            nc.tensor.matmul(
                ps_g, wc1_t[kt][:, mt * 128:(mt + 1) * 128],
                h2_fm[:, kt, sl],
                start=(kt == 0), stop=(kt == F - 1))


def _mlp(nc, tc, wc1_t, wc2f8_d, wc2bf_d, h2_fm, x2_fm, out_fm,
         bc1_c, bc2_c, yT_d):
    """MLP: c1 bf16; c2 split - first C2_U8 k-pairs pure fp8 DoubleRow
    (K=256/pass), remainder bf16 (K=128/pass). All c2 weights hold
    w_c2*32; evacuation is (ps*(1/32)) + (x2 + bc2)."""
    gelu_scale = (1.0 / 16.0) if C1DR else 1.0
    n_jobs = C2_U8 + (24 - 2 * C2_U8)
    with (
        tc.tile_pool(name="wc2p", bufs=1) as wc2_pool,
        tc.tile_pool(name="mlpc1", bufs=2, space="PSUM") as c1_ps,
        tc.tile_pool(name="mlpout", bufs=1, space="PSUM") as mo_ps,
        tc.tile_pool(name="gp8", bufs=3) as g8_pool,
        tc.tile_pool(name="gpb", bufs=3) as gb_pool,
        tc.tile_pool(name="x2bp", bufs=1) as x2b_pool,
    ):
        wc2f8_t = []
        for u in range(C2_U8):
            wt = wc2_pool.tile([128, 2 * C], f8, tag=f"wc2f{u}",
                               name=f"wc2f{u}")
            nc.sync.dma_start(wt, wc2f8_d.ap()[u * 128:(u + 1) * 128, :])
            wc2f8_t.append(wt)
        wc2bf_t = {}
        for mt in range(2 * C2_U8, 24):
            r = mt - 2 * C2_U8
            wt = wc2_pool.tile([128, C], bf16, tag=f"wc2b{mt}",
                               name=f"wc2b{mt}")
            nc.sync.dma_start(wt, wc2bf_d.ap()[r * 128:(r + 1) * 128, :])
            wc2bf_t[mt] = wt

        for c in range(NCH):
            sl = _chunk(c)
            with nc.named_scope(f"mlp_c{c}"):
                ps_out = [mo_ps.tile([128, CH], f32, tag=f"mo{ct}",
                                     name=f"mo{ct}")
                          for ct in range(F)]
                x2b = []
                for ct in range(F):
                    xt = x2b_pool.tile([128, CH], f32, tag=f"x2b{ct}",
                                       name=f"x2b{ct}")
                    nc.vector.tensor_scalar_add(
                        xt, x2_fm[:, ct, sl].bitcast(f32),
                        bc2_c[:, ct:ct + 1])
                    x2b.append(xt)

                issued = [0]

                def issue_dr(u):
                    first, last = issued[0] == 0, issued[0] == n_jobs - 1
                    for ct in range(F):
                        lhs = wc2f8_t[u].rearrange(
                            "p (a c) -> p a c", a=2)[
                            :, :, ct * 128:(ct + 1) * 128]
                        nc.tensor.matmul(
                            ps_out[ct], lhs, g8_done[u],
                            start=first, stop=last,
                            perf_mode=DR, skip_group_check=True)
                    issued[0] += 1

                def issue_bf(mt):
                    first, last = issued[0] == 0, issued[0] == n_jobs - 1
                    for ct in range(F):
                        nc.tensor.matmul(
                            ps_out[ct],
                            wc2bf_t[mt][:, ct * 128:(ct + 1) * 128],
                            gb_done[mt],
                            start=first, stop=last,
                            skip_group_check=True)
                    issued[0] += 1

                g8_done, gb_done = [], {}
                jobs = []  # (ready_mt, kind, idx)
                g_t = None
                for mt in range(24):
                    ps_g = c1_ps.tile([128, CH], f32, tag="c1ps",
                                      name="ps_g")
                    _c1_matmuls(nc, ps_g, wc1_t, h2_fm, mt, sl)
                    while jobs and jobs[0][0] <= mt - 1:
                        _, kind, idx = jobs.pop(0)
                        (issue_dr if kind == 8 else issue_bf)(idx)
                    if mt < 2 * C2_U8:
                        if mt % 2 == 0:
                            g_t = g8_pool.tile([128, 2, CH], f8, tag="g",
                                               name="g_t")
                            g8_done.append(g_t)
                        dst = g_t[:, mt % 2, :]
                        if mt % 2 == 1:
                            jobs.append((mt, 8, mt // 2))
                    else:
                        gbt = gb_pool.tile([128, CH], bf16, tag="gb",
                                           name="gbt")
                        gb_done[mt] = gbt
                        dst = gbt
                        jobs.append((mt, 16, mt))
                    nc.scalar.activation(
                        dst, ps_g, AF.Gelu,
                        bias=bc1_c[:, mt:mt + 1], scale=gelu_scale)
                for _, kind, idx in jobs:
                    (issue_dr if kind == 8 else issue_bf)(idx)
                for ct in range(F):
                    nc.vector.scalar_tensor_tensor(
                        out_fm[:, ct, sl], ps_out[ct], 1.0 / 32.0,
                        x2b[ct], ALU.mult, ALU.add)
                with nc.named_scope(f"store_c{c}"):
                    for ct in range(F):
                        nc.sync.dma_start(
                            yT_d.ap()[ct * 128:(ct + 1) * 128, sl],
                            out_fm[:, ct, sl])


def _attn_pair(nc, pools, kf, qf, v1, attn_out, tri, ones_row, f, c):
    """Attention for head pair (2f, 2f+1) on query chunk c.

    Score matmuls for the two heads (K=64, PE row groups 0-63 / 64-127)
    are issued adjacently so they run concurrently; both heads' scores
    share one [128, 2, CH] PSUM tile (adjacent banks) so exp covers both
    in a single ACT instruction. Diagonal tiles use causally-trimmed
    moving widths; masks are one [128,128] DVE multiply per head. AV is
    pipelined one kt behind scores/exp.
    """
    s_ps, y_ps, bc_ps, exp_pool, rr_pool, bcsb_pool = pools
    sl = _chunk(c)
    # key tiles: fully-causal tiles then the 4 partial (diagonal) tiles;
    # q0 = first valid query offset within the chunk
    kts = [(kt, 0, False) for kt in range(4 * c)]
    kts += [(4 * c + j, 128 * j, True) for j in range(4)]
    n_kt = len(kts)
    ps_y = [y_ps.tile([65, CH], f32, tag=f"y{hl}", name=f"ps_y{hl}")
            for hl in range(2)]

    def issue_av(i, kt, q0, expt):
        for hl in range(2):
            h_idx = 2 * f + hl
            nc.tensor.matmul(
                ps_y[hl][:, q0:CH],
                v1[:, kt, h_idx * 65:h_idx * 65 + 65],
                expt[:, hl, q0:CH],
                start=(i == 0), stop=(i == n_kt - 1))

    pend = []
    for i, (kt, q0, partial) in enumerate(kts):
        ps_s = s_ps.tile([128, 2, CH], f32, tag="s", name="ps_s")
        for hl in range(2):
            base = hl * 64
            nc.tensor.matmul(
                ps_s[:, hl, q0:CH],
                kf[base:base + 64, kt * 128:(kt + 1) * 128],
                qf[base:base + 64, c * CH + q0:(c + 1) * CH],
                start=True, stop=True)
        expt = exp_pool.tile([128, 2, CH], bf16, tag="e", name="expt")
        nc.scalar.activation(expt[:, :, q0:CH], ps_s[:, :, q0:CH], AF.Exp)
        if partial:
            # diagonal tile: mask the partial [128,128] sub-block
            for hl in range(2):
                nc.vector.tensor_mul(
                    expt[:, hl, q0:q0 + 128], expt[:, hl, q0:q0 + 128],
                    tri)
        pend.append((i, kt, q0, expt))
        if len(pend) > 3:
            issue_av(*pend.pop(0))
    while pend:
        issue_av(*pend.pop(0))

    for hl in range(2):
        base = hl * 64
        den = rr_pool.tile([1, CH], f32, tag=f"dn{hl}", name="den")
        nc.vector.tensor_copy(den, ps_y[hl][64:65, :])
        recip = rr_pool.tile([1, CH], f32, tag=f"rr{hl}", name="recip")
        nc.vector.reciprocal_approx_fast(out=recip, in_=den)
        bc = bcsb_pool.tile([64, CH], f32, tag=f"bc{hl}", name="bc")
        nc.gpsimd.partition_broadcast(bc, recip)
        nc.vector.tensor_mul(
            attn_out[base:base + 64, f, sl], ps_y[hl][0:64, :], bc)


def _get_nc():
    global _NC_CACHE
    if _NC_CACHE is None:
        _NC_CACHE = _build()
    return _NC_CACHE


def _prep_shared(inputs):
    """Host-side weight prep: LN folding, bias folds, bf16 casts,
    column-major bias layouts."""
    fd = np.float64
    lnw = np.asarray(inputs["ln_w"], fd)
    lnb = np.asarray(inputs["ln_b"], fd)
    wqkv = np.asarray(inputs["w_qkv"], fd)
    bqkv = np.asarray(inputs["b_qkv"], fd)
    wout = np.asarray(inputs["w_out"], fd)
    bout = np.asarray(inputs["b_out"], fd)
    wc1 = np.asarray(inputs["w_c1"], fd)
    bc1 = np.asarray(inputs["b_c1"], fd)
    wc2 = np.asarray(inputs["w_c2"], fd)
    bc2 = np.asarray(inputs["b_c2"], fd)

    wqkv_f = lnw[:, None] * wqkv
    bqkv_f = lnb @ wqkv + bqkv
    wc1_f = lnw[:, None] * wc1
    bc1_f = lnb @ wc1 + bc1
    bq = bqkv_f[C:2 * C]           # k-bias dropped (softmax shift inv.)
    bv = bqkv_f[2 * C:3 * C]
    bout_f = bv @ wout + bout      # v-bias folded (attn rows sum to 1)

    def pc(b, cols):
        return np.ascontiguousarray(
            b.reshape(cols, 128).T.astype(np.float32))

    bf = ml_dtypes.bfloat16
    f8np = ml_dtypes.float8_e4m3

    def hilo(w):
        """fp8 hi + residual lo (lo unscaled: fp8 subnormals capture it)."""
        w32 = np.asarray(w, np.float32)
        hi = w32.astype(f8np)
        lo = (w32 - hi.astype(np.float32)).astype(f8np)
        return hi, lo

    out = {
        "w_qkv": np.ascontiguousarray(wqkv_f.astype(np.float32)),
        "w_out": np.ascontiguousarray(wout.astype(bf)),
        "bq_pc": pc(bq, F),
        "bout_pc": pc(bout_f, F),
        "bc1_pc": pc(bc1_f, 24),
        "bc2_pc": pc(bc2, F),
    }
    if C1DR:
        # k-pair layout [3*128, 2*M3]: (j, p, i, m) <- w16[(2j+i)*128+p, m]
        w16 = (wc1_f * 16.0).reshape(3, 2, 128, M3).transpose(0, 2, 1, 3)
        hi, lo = hilo(w16.reshape(3 * 128, 2 * M3))
        out["w_c1h"] = np.ascontiguousarray(hi)
        out["w_c1l"] = np.ascontiguousarray(lo)
    else:
        out["w_c1"] = np.ascontiguousarray(wc1_f.astype(bf))
    w32 = wc2 * 32.0
    if C2_U8:
        # k-pair layout [U8*128, 2*C]: (u, p, i, c) <- w32[(2u+i)*128+p, c]
        pairs = (w32[: C2_U8 * 256].reshape(C2_U8, 2, 128, C)
                 .transpose(0, 2, 1, 3).reshape(C2_U8 * 128, 2 * C))
        out["w_c2f8"] = np.ascontiguousarray(
            pairs.astype(np.float32).astype(f8np))
    if C2_U8 < 12:
        out["w_c2bf"] = np.ascontiguousarray(
            w32[C2_U8 * 256:].astype(bf))
    return out


def run(inputs, trace=False, debug_stage=None):
    if debug_stage:
        nc = _build(debug_stage)
    else:
        nc = _get_nc()
    xs = np.asarray(inputs["x"], dtype=np.float32)
    assert xs.shape == (B, T, C), xs.shape
    shared = _prep_shared(inputs)
    in_maps = [dict(shared, xT=np.ascontiguousarray(xs[c].T))
               for c in range(B)]
    res = bass_utils.run_bass_kernel_spmd(
        nc, in_maps, core_ids=list(range(B)), trace=trace)
    out = np.stack([np.ascontiguousarray(r["yT"].T)
                    for r in res.results], axis=0)
    if debug_stage:
        dbg = np.stack([np.ascontiguousarray(r["yD"].T)
                        for r in res.results], axis=0)
        return out, res, dbg
    return out, res


def kernel(**inputs):
    out, _ = run(inputs, trace=False)
    return out



# revision 34
# speedup vs baseline: 1.1194x; 1.1194x over previous
"""Trainium2 Bass kernel for nn_Block_50706383897045 (dense transformer block).

Strategy: data-parallel over batch - B=8 equals n_cores=8, one batch element
per core, no collectives. Per core the full block (LN -> QKV -> causal
attention -> out-proj -> residual -> LN -> MLP(gelu) -> residual) runs on a
[T=1024, C=768] slice.

v2 design notes:
- Host prep: x pre-transposed to feature-major (no on-chip transposes);
  ln_w/ln_b folded into w_qkv/w_c1 (both LNs share params); k-bias dropped
  (softmax shift invariance); v-bias folded into b_out (attn rows sum to 1);
  weights cast to bf16 (halves DMA + SBUF).
- LN: stats via ones-matmuls on PE, rstd via ACT Sqrt + DVE approx
  reciprocal, mean/rstd broadcast via K=1 PE matmuls in f32r (bf16 rows
  here would scale h per-token by ~0.4% and get exp-amplified to ~30%
  attention-weight error - keep them f32r), normalize = 2 DVE ops.
- Attention: h/k/q/scores in f32r (precision: score errors multiply
  through exp); causal trim (partial diagonal tiles use reduced moving
  width); the two heads of a feature tile issue score matmuls adjacently
  -> PE row-group concurrency (K=64, tile_position from base partitions);
  one exp per kt covers both heads ([128,2,CH] PSUM, adjacent banks);
  exp -> bf16; mask = one [128,128] DVE multiply per diagonal tile;
  denominator from a ones-column in V; reciprocal via DVE approx op
  (must read from SBUF, not PSUM); broadcast via gpsimd; attn@v bf16.
  KQ(f+1) software-pipelined under attention(f).
- PSUM evacuations fused with bias+residual via scalar_tensor_tensor.
- v1 1070785ns (harness) / 634us (local) -> v2 365620ns local.
"""
import sys

sys.path.insert(0, "/opt/trn_rl_repo")

import numpy as np
import ml_dtypes

import concourse.bass as bass
import concourse.bacc as bacc
import concourse.mybir as mybir
import concourse.tile as tile
from concourse import bass_utils

AF = mybir.ActivationFunctionType
ALU = mybir.AluOpType
f32 = mybir.dt.float32
f32r = mybir.dt.float32r
bf16 = mybir.dt.bfloat16
f8 = mybir.dt.float8e4
DR = mybir.MatmulPerfMode.DoubleRow

# PE throughput is 1 moving-row/cycle regardless of dtype; fp8 DoubleRow's
# win is K=256 per pass (half the row-passes), so only PURE fp8 (no hi/lo
# residual) is faster than bf16. Error budget allows it on part of c2 only:
# the first C2_U8 k-pairs (of 12) of c2 run fp8-DR (g + w_c2 quantized to
# e4m3, ~2% rms each); the rest stays bf16. All c2 weights are scaled x32
# (fp8 subnormal avoidance), descaled in the evacuation.
C1DR = False  # hi/lo c1 measured: same speed as bf16, extra noise. Keep off.
C2_U8 = 12    # k-pairs of c2 in fp8-DR (0..12); 12 = full fp8 c2

B, T, C, H, D = 8, 1024, 768, 12, 64
F = C // 128      # 6 feature tiles of the residual stream
NT = T // 128     # 8 token tiles
CH = 512          # token chunk (fp32 moving-operand max)
NCH = T // CH     # 2
M3 = 4 * C        # 3072 MLP hidden
EPS = 1e-5

_NC_CACHE = None


def _chunk(c):
    return slice(c * CH, (c + 1) * CH)


def _ln(nc, tc, pools, src, dst, ones_col, ones_row, eps_t, sq_on_act=True,
        chunks=None):
    """LayerNorm (params pre-folded into weights): dst = (src-mu)*rstd.
    src f32r [128, F, T], dst bf16 [128, F, T]. Stats via ones-matmuls,
    rstd via ACT Rsqrt, broadcast via K=1 PE matmuls."""
    ln_ps, sq_pool, row_pool, tmp_pool = pools
    for c in (range(NCH) if chunks is None else chunks):
        sl = _chunk(c)
        ps_sum = ln_ps.tile([1, CH], f32, tag="lnsum", name="ps_sum")
        ps_sq = ln_ps.tile([1, CH], f32, tag="lnsq", name="ps_sq")
        hc = CH // 2
        for f in range(F):
            # square split ACT/DVE halves so sq_t is ready ~2x sooner
            sq_t = sq_pool.tile([128, CH], f32r, tag="ln_sq", name="sq_t")
            nc.scalar.activation(sq_t[:, 0:hc], src[:, f, sl][:, 0:hc],
                                 AF.Square)
            nc.vector.tensor_mul(sq_t[:, hc:CH], src[:, f, sl][:, hc:CH],
                                 src[:, f, sl][:, hc:CH])
            nc.tensor.matmul(ps_sum, ones_col, src[:, f, sl],
                             start=(f == 0), stop=(f == F - 1))
            nc.tensor.matmul(ps_sq, ones_col, sq_t,
                             start=(f == 0), stop=(f == F - 1))
        mean = row_pool.tile([1, CH], f32, tag="ln_ra", name="mean")
        nc.vector.tensor_scalar_mul(mean, ps_sum, 1.0 / C)
        musq = row_pool.tile([1, CH], f32, tag="ln_rb", name="musq")
        nc.vector.tensor_mul(musq, mean, mean)
        var = row_pool.tile([1, CH], f32, tag="ln_rc", name="var")
        nc.vector.scalar_tensor_tensor(
            var, ps_sq, 1.0 / C, musq, ALU.mult, ALU.subtract)
        std = row_pool.tile([1, CH], f32, tag="ln_rb", name="std")
        nc.scalar.activation(std, var, AF.Sqrt, bias=eps_t)
        rstd = row_pool.tile([1, CH], f32, tag="ln_rc", name="rstd")
        nc.vector.reciprocal_approx_fast(out=rstd, in_=std)
        nmrs = row_pool.tile([1, CH], f32r, tag="ln_nm", name="nmrs")
        nc.vector.scalar_tensor_tensor(
            nmrs, mean, -1.0, rstd, ALU.mult, ALU.mult)
        rstd_b = row_pool.tile([1, CH], f32r, tag="ln_rb", name="rstd_b")
        nc.vector.tensor_copy(rstd_b, rstd)
        ps_rs = ln_ps.tile([128, CH], f32, tag="lnbc_rs", name="ps_rs")
        nc.tensor.matmul(ps_rs, ones_row, rstd_b, start=True, stop=True)
        ps_nm = ln_ps.tile([128, CH], f32, tag="lnbc_nm", name="ps_nm")
        nc.tensor.matmul(ps_nm, ones_row, nmrs, start=True, stop=True)
        for f in range(F):
            tmp = tmp_pool.tile([128, CH], f32, tag="ln_tmp", name="tmp")
            nc.vector.tensor_mul(tmp, src[:, f, sl].bitcast(f32), ps_rs)
            nc.vector.tensor_add(dst[:, f, sl], tmp, ps_nm)


def _build(debug_stage=None):
    nc = bacc.Bacc("TRN2", target_bir_lowering=False, debug=False,
                   num_devices=8)

    xT_d = nc.dram_tensor("xT", [C, T], f32, kind="ExternalInput")
    wqkv_d = nc.dram_tensor("w_qkv", [C, 3 * C], f32, kind="ExternalInput")
    wout_d = nc.dram_tensor("w_out", [C, C], bf16, kind="ExternalInput")
    if C1DR:
        # k-pair layout: [3*128, 2*M3]; row j*128+p, col i*M3+m holds
        # w_c1[(2j+i)*128+p, m] * 16 (hi) / its fp8 residual (lo)
        wc1_d = nc.dram_tensor("w_c1h", [3 * 128, 2 * M3], f8,
                               kind="ExternalInput")
        wc1l_d = nc.dram_tensor("w_c1l", [3 * 128, 2 * M3], f8,
                                kind="ExternalInput")
    else:
        wc1_d = nc.dram_tensor("w_c1", [C, M3], bf16, kind="ExternalInput")
        wc1l_d = None
    # c2 weights: first C2_U8 k-pairs as fp8 pair-layout, rest bf16 rows;
    # both hold w_c2 * 32
    wc2_d = (nc.dram_tensor("w_c2f8", [C2_U8 * 128, 2 * C], f8,
                            kind="ExternalInput") if C2_U8 else None)
    wc2l_d = (nc.dram_tensor("w_c2bf", [(24 - 2 * C2_U8) * 128, C], bf16,
                             kind="ExternalInput") if C2_U8 < 12 else None)
    bq_d = nc.dram_tensor("bq_pc", [128, F], f32, kind="ExternalInput")
    bout_d = nc.dram_tensor("bout_pc", [128, F], f32, kind="ExternalInput")
    bc1_d = nc.dram_tensor("bc1_pc", [128, 24], f32, kind="ExternalInput")
    bc2_d = nc.dram_tensor("bc2_pc", [128, F], f32, kind="ExternalInput")
    yT_d = nc.dram_tensor("yT", [C, T], f32, kind="ExternalOutput")
    yD_d = (nc.dram_tensor("yD", [C, T], f32, kind="ExternalOutput")
            if debug_stage else None)

    with tile.TileContext(nc) as tc:
        _kernel_body(nc, tc, xT_d, wqkv_d, wout_d, wc1_d, wc1l_d,
                     wc2_d, wc2l_d, bq_d, bout_d, bc1_d, bc2_d, yT_d,
                     debug_stage, yD_d)
    nc.compile()
    return nc


def _debug_dump(nc, tc, src_t, yD_d, cast=True):
    """Copy a [128, F, T] tile to the yD debug output."""
    with tc.tile_pool(name="dbg", bufs=2) as dbg_pool:
        for ct in range(F):
            for c in range(NCH):
                sl = _chunk(c)
                t = dbg_pool.tile([128, CH], f32, tag="dbg", name="dbg")
                nc.vector.tensor_copy(t, src_t[:, ct, sl])
                nc.sync.dma_start(
                    yD_d.ap()[ct * 128:(ct + 1) * 128, sl], t)


def _kernel_body(nc, tc, xT_d, wqkv_d, wout_d, wc1_d, wc1l_d,
                 wc2_d, wc2l_d, bq_d, bout_d, bc1_d, bc2_d, yT_d,
                 debug_stage=None, yD_d=None):
    with tc.tile_pool(name="persist", bufs=1) as persist:
        ones_col = persist.tile([128, 1], f32r)
        nc.vector.memset(ones_col.bitcast(f32), 1.0)
        ones_row = persist.tile([1, 128], f32r)
        nc.vector.memset(ones_row.bitcast(f32), 1.0)
        eps_t = persist.tile([1, 1], f32)
        nc.vector.memset(eps_t, EPS)
        # lower-triangular keep mask (tri[p, q] = 1 iff q >= p)
        tri = persist.tile([128, 128], bf16)
        with tc.tile_pool(name="trif", bufs=1) as trif_pool:
            tri_f = trif_pool.tile([128, 128], f32)
            nc.vector.memset(tri_f, 1.0)
            nc.gpsimd.affine_select(
                out=tri_f, in_=tri_f, compare_op=ALU.is_ge, fill=0.0,
                base=0, pattern=[[1, 128]], channel_multiplier=-1)
            nc.vector.tensor_copy(tri, tri_f)
        bq_c = persist.tile([128, F], f32)
        bout_c = persist.tile([128, F], f32)
        bc1_c = persist.tile([128, 24], f32)
        bc2_c = persist.tile([128, F], f32)


        with (
            tc.tile_pool(name="resid", bufs=1) as resid_pool,
            tc.tile_pool(name="hpool", bufs=1) as h_pool,
            tc.tile_pool(name="aopool", bufs=1) as ao_pool,
            tc.tile_pool(name="woutp", bufs=1) as wout_pool,
            tc.tile_pool(name="wc1p", bufs=1) as wc1_pool,
        ):
            x_fm = resid_pool.tile([128, F, T], f32r, tag="x_slot",
                                   name="x_fm")
            h_fm = h_pool.tile([128, F, T], f32r, tag="h_slot", name="h_fm")
            attn_out = ao_pool.tile([128, F, T], bf16, tag="attn_out",
                                    name="attn_out")

            # ---- input + weight DMAs (issued up front, in need order) ----
            for c in range(NCH):
                for f in range(F):
                    nc.sync.dma_start(
                        x_fm[:, f, _chunk(c)],
                        xT_d.ap().bitcast(f32r)
                        [f * 128:(f + 1) * 128, _chunk(c)])
            nc.sync.dma_start(bq_c, bq_d.ap())
            nc.sync.dma_start(bout_c, bout_d.ap())
            nc.sync.dma_start(bc1_c, bc1_d.ap())
            nc.sync.dma_start(bc2_c, bc2_d.ap())

            with (
                tc.tile_pool(name="wkq", bufs=1) as wkq_pool,
                tc.tile_pool(name="v1pool", bufs=1) as v1_pool,
            ):
                wv_pool = tc.alloc_tile_pool(name="wv", bufs=1)
                wv_t, wkq_t, wout_t, wc1_t = [], [], [], []
                # DMA issue order = need order: wv (~45us), wkq (~75us),
                # wout (~215us), wc1 (~230us)
                for kt in range(F):
                    wt = wv_pool.tile([128, C], f32r, tag=f"wv{kt}",
                                      name=f"wv{kt}")
                    nc.sync.dma_start(
                        wt, wqkv_d.ap().bitcast(f32r)
                        [kt * 128:(kt + 1) * 128, 2 * C:3 * C])
                    wv_t.append(wt)
                for kt in range(F):
                    wt = wkq_pool.tile([128, 2 * C], f32r, tag=f"wkq{kt}",
                                       name=f"wkq{kt}")
                    nc.sync.dma_start(
                        wt, wqkv_d.ap().bitcast(f32r)
                        [kt * 128:(kt + 1) * 128, 0:2 * C])
                    wkq_t.append(wt)
                for kt in range(F):
                    wt = wout_pool.tile([128, C], bf16, tag=f"wout{kt}",
                                        name=f"wout{kt}")
                    nc.sync.dma_start(
                        wt, wout_d.ap()[kt * 128:(kt + 1) * 128, :])
                    wout_t.append(wt)
                if C1DR:
                    for j in range(3):
                        for lbl, dram in (("h", wc1_d), ("l", wc1l_d)):
                            wt = wc1_pool.tile(
                                [128, 2 * M3], f8, tag=f"wc1{lbl}{j}",
                                name=f"wc1{lbl}{j}")
                            nc.sync.dma_start(
                                wt, dram.ap()[j * 128:(j + 1) * 128, :])
                            wc1_t.append(wt)
                else:
                    for kt in range(F):
                        wt = wc1_pool.tile([128, M3], bf16, tag=f"wc1{kt}",
                                           name=f"wc1{kt}")
                        nc.sync.dma_start(
                            wt, wc1_d.ap()[kt * 128:(kt + 1) * 128, :])
                        wc1_t.append(wt)

                # V with appended ones column per head (softmax denominator)
                v1 = v1_pool.tile([128, NT, H * 65], bf16, tag="v1",
                                  name="v1")
                nc.vector.memset(
                    v1.rearrange("p t (h m) -> p t h m", m=65)
                    [:, :, :, 64:65], 1.0)

                # ---- LN1 then V ----
                with (
                    tc.tile_pool(name="lnps", bufs=1, space="PSUM") as ln_ps,
                    tc.tile_pool(name="ln_sq", bufs=1) as sq_pool,
                    tc.tile_pool(name="ln_rows", bufs=1) as row_pool,
                    tc.tile_pool(name="ln_tmp", bufs=2) as tmp_pool,
                    tc.tile_pool(name="vps", bufs=3, space="PSUM") as v_ps,
                ):
                    with nc.named_scope("ln1"):
                        _ln(nc, tc, (ln_ps, sq_pool, row_pool,
                             tmp_pool),
                            x_fm, h_fm, ones_col, ones_row, eps_t)
                    with nc.named_scope("qkv_v"):
                        for t in range(NT):  # noqa: E501
                            for half in range(2):
                                ps_v = v_ps.tile([128, 384], f32, tag="vps",
                                                 name="ps_v")
                                c0 = half * 384
                                for kt in range(F):
                                    nc.tensor.matmul(
                                        ps_v,
                                        h_fm[:, kt, t * 128:(t + 1) * 128],
                                        wv_t[kt][:, c0:c0 + 384],
                                        start=(kt == 0), stop=(kt == F - 1))
                                dst = (v1[:, t, :]
                                       .rearrange("p (h m) -> p h m", m=65)
                                       [:, half * 6:(half + 1) * 6, 0:64])
                                nc.scalar.copy(
                                    dst,
                                    ps_v.rearrange("p (h m) -> p h m",
                                                   m=64))
                wv_pool.release()

                # ---- per feature tile: K,Q then attention (both chunks)
                with (
                    tc.tile_pool(name="kqf", bufs=2) as kqf_pool,
                    tc.tile_pool(name="sps", bufs=2, space="PSUM") as s_ps,
                    tc.tile_pool(name="yps", bufs=2, space="PSUM") as y_ps,
                    tc.tile_pool(name="bcps", bufs=1, space="PSUM") as bc_ps,
                    tc.tile_pool(name="expp", bufs=4) as exp_pool,
                    tc.tile_pool(name="rrow", bufs=1) as rr_pool,
                    tc.tile_pool(name="bcsb", bufs=1) as bcsb_pool,
                ):
                    def kq_phase(f):
                        """Emit K,Q matmuls + evacs for feature tile f."""
                        kf = kqf_pool.tile([128, T], f32r, tag="kf",
                                           name=f"kf{f}")
                        qf = kqf_pool.tile([128, T], f32r, tag="qf",
                                           name=f"qf{f}")
                        with nc.named_scope(f"kq_{f}"):
                            for dst_t, col0, isq in (
                                    (kf, f * 128, False),
                                    (qf, C + f * 128, True)):
                                for c in range(NCH):
                                    sl = _chunk(c)
                                    ps = s_ps.tile([128, 2, CH], f32,
                                                   tag="s", name="kq_ps")
                                    ps = ps[:, 0, :]
                                    for kt in range(F):
                                        nc.tensor.matmul(
                                            ps,
                                            wkq_t[kt][:, col0:col0 + 128],
                                            h_fm[:, kt, sl],
                                            start=(kt == 0),
                                            stop=(kt == F - 1))
                                    if isq:
                                        nc.vector.tensor_scalar_add(
                                            dst_t[:, sl],
                                            ps, bq_c[:, f:f + 1])
                                    else:
                                        nc.scalar.copy(
                                            dst_t[:, sl], ps)
                        return kf, qf

                    # software pipeline: KQ(f+1) issues before attn(f) so
                    # its PE matmuls cover the kf/qf evacuation latency
                    kqf_t = kq_phase(0)
                    for f in range(F):
                        nxt = kq_phase(f + 1) if f + 1 < F else None
                        kf, qf = kqf_t
                        for c in range(NCH):
                            with nc.named_scope(f"attn_f{f}_c{c}"):
                                _attn_pair(nc, (s_ps, y_ps, bc_ps, exp_pool,
                                                rr_pool, bcsb_pool),
                                           kf, qf, v1, attn_out,
                                           tri, ones_row, f, c)
                        kqf_t = nxt

            if debug_stage == 'h':
                _debug_dump(nc, tc, h_fm, yD_d)
            if debug_stage == 'attn':
                _debug_dump(nc, tc, attn_out, yD_d)

            x2_pool = tc.alloc_tile_pool(name="x2p", bufs=1)
            x2_fm = x2_pool.tile([128, F, T], f32r, tag="x2",
                                 name="x2_fm")
            h2_fm = h_pool.tile([128, F, T], f8 if C1DR else bf16,
                                tag="h_slot", name="h2_fm")
            with (
                tc.tile_pool(name="ops2", bufs=3, space="PSUM") as o2_ps,
                tc.tile_pool(name="ln2ps", bufs=1, space="PSUM") as ln2_ps,
                tc.tile_pool(name="ln2_sq", bufs=1) as sq2_pool,
                tc.tile_pool(name="ln2_rows", bufs=1) as row2_pool,
                tc.tile_pool(name="ln2_tmp", bufs=2) as tmp2_pool,
            ):
                for c in range(NCH):
                    sl = _chunk(c)
                    with nc.named_scope(f"out_proj_c{c}"):
                        for ct in range(F):
                            ps = o2_ps.tile([128, CH], f32, tag="o",
                                            name="o_ps")
                            for kt in range(F):
                                nc.tensor.matmul(
                                    ps,
                                    wout_t[kt][:, ct * 128:(ct + 1) * 128],
                                    attn_out[:, kt, sl],
                                    start=(kt == 0), stop=(kt == F - 1))
                            nc.vector.scalar_tensor_tensor(
                                x2_fm[:, ct, sl], ps,
                                bout_c[:, ct:ct + 1],
                                x_fm[:, ct, sl].bitcast(f32),
                                ALU.add, ALU.add)
                    with nc.named_scope(f"ln2_c{c}"):
                        _ln(nc, tc, (ln2_ps, sq2_pool, row2_pool,
                             tmp2_pool),
                            x2_fm, h2_fm, ones_col, ones_row, eps_t,
                            sq_on_act=False, chunks=[c])

            if True:
                if debug_stage == 'x2':
                    _debug_dump(nc, tc, x2_fm, yD_d)

                if debug_stage == 'h2':
                    _debug_dump(nc, tc, h2_fm, yD_d)

                # ---- MLP ----
                out_fm = resid_pool.tile([128, F, T], f32, tag="x_slot",
                                         name="out_fm")
                _mlp(nc, tc, wc1_t, wc2_d, wc2l_d, h2_fm, x2_fm,
                     out_fm, bc1_c, bc2_c, yT_d)
            x2_pool.release()


def _c1_matmuls(nc, ps_g, wc1_t, h2_fm, mt, sl):
    """Emit the c1 accumulation group for output tile mt."""
    if C1DR:
        # wc1_t = [h0, l0, h1, l1, h2, l2]; ps = sum_j Whi_j.T2 @ h_j
        # + sum_j Wlo_j.T2 @ h_j  (DoubleRow, K=256 per matmul)
        n = 6
        i = 0
        for lbl in range(2):  # hi then lo
            for j in range(3):
                wt = wc1_t[2 * j + lbl]
                lhs = wt.rearrange("p (a m) -> p a m", a=2)[
                    :, :, mt * 128:(mt + 1) * 128]
                nc.tensor.matmul(
                    ps_g, lhs, h2_fm[:, 2 * j:2 * j + 2, sl],
                    start=(i == 0), stop=(i == n - 1), perf_mode=DR)
                i += 1
    else:
        for kt in range(F):
            nc.tensor.matmul(
                ps_g, wc1_t[kt][:, mt * 128:(mt + 1) * 128],
                h2_fm[:, kt, sl],
                start=(kt == 0), stop=(kt == F - 1))


def _mlp(nc, tc, wc1_t, wc2f8_d, wc2bf_d, h2_fm, x2_fm, out_fm,
         bc1_c, bc2_c, yT_d):
    """MLP: c1 bf16; c2 split - first C2_U8 k-pairs pure fp8 DoubleRow
    (K=256/pass), remainder bf16 (K=128/pass). All c2 weights hold
    w_c2*32; evacuation is (ps*(1/32)) + (x2 + bc2)."""
    gelu_scale = (1.0 / 16.0) if C1DR else 1.0
    n_jobs = C2_U8 + (24 - 2 * C2_U8)
    with (
        tc.tile_pool(name="wc2p", bufs=1) as wc2_pool,
        tc.tile_pool(name="mlpc1", bufs=2, space="PSUM") as c1_ps,
        tc.tile_pool(name="mlpout", bufs=1, space="PSUM") as mo_ps,
        tc.tile_pool(name="gp8", bufs=3) as g8_pool,
        tc.tile_pool(name="gpb", bufs=3) as gb_pool,
        tc.tile_pool(name="x2bp", bufs=1) as x2b_pool,
    ):
        wc2f8_t = []
        for u in range(C2_U8):
            wt = wc2_pool.tile([128, 2 * C], f8, tag=f"wc2f{u}",
                               name=f"wc2f{u}")
            nc.sync.dma_start(wt, wc2f8_d.ap()[u * 128:(u + 1) * 128, :])
            wc2f8_t.append(wt)
        wc2bf_t = {}
        for mt in range(2 * C2_U8, 24):
            r = mt - 2 * C2_U8
            wt = wc2_pool.tile([128, C], bf16, tag=f"wc2b{mt}",
                               name=f"wc2b{mt}")
            nc.sync.dma_start(wt, wc2bf_d.ap()[r * 128:(r + 1) * 128, :])
            wc2bf_t[mt] = wt

        for c in range(NCH):
            sl = _chunk(c)
            with nc.named_scope(f"mlp_c{c}"):
                ps_out = [mo_ps.tile([128, CH], f32, tag=f"mo{ct}",
                                     name=f"mo{ct}")
                          for ct in range(F)]
                x2b = []
                for ct in range(F):
                    xt = x2b_pool.tile([128, CH], f32, tag=f"x2b{ct}",
                                       name=f"x2b{ct}")
                    nc.vector.tensor_scalar_add(
                        xt, x2_fm[:, ct, sl].bitcast(f32),
                        bc2_c[:, ct:ct + 1])
                    x2b.append(xt)

                issued = [0]

                def issue_dr(u):
                    first, last = issued[0] == 0, issued[0] == n_jobs - 1
                    for ct in range(F):
                        lhs = wc2f8_t[u].rearrange(
                            "p (a c) -> p a c", a=2)[
                            :, :, ct * 128:(ct + 1) * 128]
                        nc.tensor.matmul(
                            ps_out[ct], lhs, g8_done[u],
                            start=first, stop=last,
                            perf_mode=DR, skip_group_check=True)
                    issued[0] += 1

                def issue_bf(mt):
                    first, last = issued[0] == 0, issued[0] == n_jobs - 1
                    for ct in range(F):
                        nc.tensor.matmul(
                            ps_out[ct],
                            wc2bf_t[mt][:, ct * 128:(ct + 1) * 128],
                            gb_done[mt],
                            start=first, stop=last,
                            skip_group_check=True)
                    issued[0] += 1

                g8_done, gb_done = [], {}
                jobs = []  # (ready_mt, kind, idx)
                g_t = None
                for mt in range(24):
                    ps_g = c1_ps.tile([128, CH], f32, tag="c1ps",
                                      name="ps_g")
                    _c1_matmuls(nc, ps_g, wc1_t, h2_fm, mt, sl)
                    while jobs and jobs[0][0] <= mt - 1:
                        _, kind, idx = jobs.pop(0)
                        (issue_dr if kind == 8 else issue_bf)(idx)
                    if mt < 2 * C2_U8:
                        if mt % 2 == 0:
                            g_t = g8_pool.tile([128, 2, CH], f8, tag="g",
                                               name="g_t")
                            g8_done.append(g_t)
                        dst = g_t[:, mt % 2, :]
                        if mt % 2 == 1:
                            jobs.append((mt, 8, mt // 2))
                    else:
                        gbt = gb_pool.tile([128, CH], bf16, tag="gb",
                                           name="gbt")
                        gb_done[mt] = gbt
                        dst = gbt
                        jobs.append((mt, 16, mt))
                    nc.scalar.activation(
                        dst, ps_g, AF.Gelu,
                        bias=bc1_c[:, mt:mt + 1], scale=gelu_scale)
                for _, kind, idx in jobs:
                    (issue_dr if kind == 8 else issue_bf)(idx)
                for ct in range(F):
                    nc.vector.scalar_tensor_tensor(
                        out_fm[:, ct, sl], ps_out[ct], 1.0 / 32.0,
                        x2b[ct], ALU.mult, ALU.add)
                with nc.named_scope(f"store_c{c}"):
                    for ct in range(F):
                        nc.sync.dma_start(
                            yT_d.ap()[ct * 128:(ct + 1) * 128, sl],
                            out_fm[:, ct, sl])


def _attn_pair(nc, pools, kf, qf, v1, attn_out, tri, ones_row, f, c):
    """Attention for head pair (2f, 2f+1) on query chunk c.

    Score matmuls for the two heads (K=64, PE row groups 0-63 / 64-127)
    are issued adjacently so they run concurrently; both heads' scores
    share one [128, 2, CH] PSUM tile (adjacent banks) so exp covers both
    in a single ACT instruction. Diagonal tiles use causally-trimmed
    moving widths; masks are one [128,128] DVE multiply per head. AV is
    pipelined one kt behind scores/exp.
    """
    s_ps, y_ps, bc_ps, exp_pool, rr_pool, bcsb_pool = pools
    sl = _chunk(c)
    # key tiles: fully-causal tiles then the 4 partial (diagonal) tiles;
    # q0 = first valid query offset within the chunk
    kts = [(kt, 0, False) for kt in range(4 * c)]
    kts += [(4 * c + j, 128 * j, True) for j in range(4)]
    n_kt = len(kts)
    ps_y = [y_ps.tile([65, CH], f32, tag=f"y{hl}", name=f"ps_y{hl}")
            for hl in range(2)]

    def issue_av(i, kt, q0, expt):
        for hl in range(2):
            h_idx = 2 * f + hl
            nc.tensor.matmul(
                ps_y[hl][:, q0:CH],
                v1[:, kt, h_idx * 65:h_idx * 65 + 65],
                expt[:, hl, q0:CH],
                start=(i == 0), stop=(i == n_kt - 1))

    pend = []
    for i, (kt, q0, partial) in enumerate(kts):
        ps_s = s_ps.tile([128, 2, CH], f32, tag="s", name="ps_s")
        for hl in range(2):
            base = hl * 64
            nc.tensor.matmul(
                ps_s[:, hl, q0:CH],
                kf[base:base + 64, kt * 128:(kt + 1) * 128],
                qf[base:base + 64, c * CH + q0:(c + 1) * CH],
                start=True, stop=True)
        expt = exp_pool.tile([128, 2, CH], bf16, tag="e", name="expt")
        nc.scalar.activation(expt[:, :, q0:CH], ps_s[:, :, q0:CH], AF.Exp)
        if partial:
            # diagonal tile: mask the partial [128,128] sub-block
            for hl in range(2):
                nc.vector.tensor_mul(
                    expt[:, hl, q0:q0 + 128], expt[:, hl, q0:q0 + 128],
                    tri)
        pend.append((i, kt, q0, expt))
        if len(pend) > 3:
            issue_av(*pend.pop(0))
    while pend:
        issue_av(*pend.pop(0))

    for hl in range(2):
        base = hl * 64
        den = rr_pool.tile([1, CH], f32, tag=f"dn{hl}", name="den")
        nc.vector.tensor_copy(den, ps_y[hl][64:65, :])
        recip = rr_pool.tile([1, CH], f32, tag=f"rr{hl}", name="recip")
        nc.vector.reciprocal_approx_fast(out=recip, in_=den)
        bc = bcsb_pool.tile([64, CH], f32, tag=f"bc{hl}", name="bc")
        nc.gpsimd.partition_broadcast(bc, recip)
        nc.vector.tensor_mul(
            attn_out[base:base + 64, f, sl], ps_y[hl][0:64, :], bc)


def _get_nc():
    global _NC_CACHE
    if _NC_CACHE is None:
        _NC_CACHE = _build()
    return _NC_CACHE


def _prep_shared(inputs):
    """Host-side weight prep: LN folding, bias folds, bf16 casts,
    column-major bias layouts."""
    fd = np.float64
    lnw = np.asarray(inputs["ln_w"], fd)
    lnb = np.asarray(inputs["ln_b"], fd)
    wqkv = np.asarray(inputs["w_qkv"], fd)
    bqkv = np.asarray(inputs["b_qkv"], fd)
    wout = np.asarray(inputs["w_out"], fd)
    bout = np.asarray(inputs["b_out"], fd)
    wc1 = np.asarray(inputs["w_c1"], fd)
    bc1 = np.asarray(inputs["b_c1"], fd)
    wc2 = np.asarray(inputs["w_c2"], fd)
    bc2 = np.asarray(inputs["b_c2"], fd)

    wqkv_f = lnw[:, None] * wqkv
    bqkv_f = lnb @ wqkv + bqkv
    wc1_f = lnw[:, None] * wc1
    bc1_f = lnb @ wc1 + bc1
    bq = bqkv_f[C:2 * C]           # k-bias dropped (softmax shift inv.)
    bv = bqkv_f[2 * C:3 * C]
    bout_f = bv @ wout + bout      # v-bias folded (attn rows sum to 1)

    def pc(b, cols):
        return np.ascontiguousarray(
            b.reshape(cols, 128).T.astype(np.float32))

    bf = ml_dtypes.bfloat16
    f8np = ml_dtypes.float8_e4m3

    def hilo(w):
        """fp8 hi + residual lo (lo unscaled: fp8 subnormals capture it)."""
        w32 = np.asarray(w, np.float32)
        hi = w32.astype(f8np)
        lo = (w32 - hi.astype(np.float32)).astype(f8np)
        return hi, lo

    out = {
        "w_qkv": np.ascontiguousarray(wqkv_f.astype(np.float32)),
        "w_out": np.ascontiguousarray(wout.astype(bf)),
        "bq_pc": pc(bq, F),
        "bout_pc": pc(bout_f, F),
        "bc1_pc": pc(bc1_f, 24),
        "bc2_pc": pc(bc2, F),
    }
    if C1DR:
        # k-pair layout [3*128, 2*M3]: (j, p, i, m) <- w16[(2j+i)*128+p, m]
        w16 = (wc1_f * 16.0).reshape(3, 2, 128, M3).transpose(0, 2, 1, 3)
        hi, lo = hilo(w16.reshape(3 * 128, 2 * M3))
        out["w_c1h"] = np.ascontiguousarray(hi)
        out["w_c1l"] = np.ascontiguousarray(lo)
    else:
        out["w_c1"] = np.ascontiguousarray(wc1_f.astype(bf))
    w32 = wc2 * 32.0
    if C2_U8:
        # k-pair layout [U8*128, 2*C]: (u, p, i, c) <- w32[(2u+i)*128+p, c]
        pairs = (w32[: C2_U8 * 256].reshape(C2_U8, 2, 128, C)
                 .transpose(0, 2, 1, 3).reshape(C2_U8 * 128, 2 * C))
        out["w_c2f8"] = np.ascontiguousarray(
            pairs.astype(np.float32).astype(f8np))
    if C2_U8 < 12:
        out["w_c2bf"] = np.ascontiguousarray(
            w32[C2_U8 * 256:].astype(bf))
    return out


def run(inputs, trace=False, debug_stage=None):
    if debug_stage:
        nc = _build(debug_stage)
    else:
        nc = _get_nc()
    xs = np.asarray(inputs["x"], dtype=np.float32)
    assert xs.shape == (B, T, C), xs.shape
    shared = _prep_shared(inputs)
    in_maps = [dict(shared, xT=np.ascontiguousarray(xs[c].T))
               for c in range(B)]
    res = bass_utils.run_bass_kernel_spmd(
        nc, in_maps, core_ids=list(range(B)), trace=trace)
    out = np.stack([np.ascontiguousarray(r["yT"].T)
                    for r in res.results], axis=0)
    if debug_stage:
        dbg = np.stack([np.ascontiguousarray(r["yD"].T)
                        for r in res.results], axis=0)
        return out, res, dbg
    return out, res


def kernel(**inputs):
    out, _ = run(inputs, trace=False)
    return out

